# revision 18
# baseline (speedup 1.0000x reference)
"""Trainium2 Bass kernel: GQA attention with KV cache (decode, Sq=4).

Problem shapes (hardcoded):
  Q [4, 4, 32, 128] f32, K [4, 8192, 8, 128] f32, V [4, 8192, 8, 128] f32,
  cache_seqlens [4] i32 in [4096, 8192].  Output [4, 4, 32, 128] f32.

Sharding: tensor-parallel over the 8 KV heads — core c owns KV head c and
its 4 grouped query heads, for all 4 batches.  Every core therefore does
identical work regardless of cache_seqlens skew.

v2 design (DMA-bound; ~10.96 MB/core of K+V at ~420 GB/s):
  - K is stored as fp8 e3m4 (x2 scale, clipped to +-15.5); Q is bf16 and
    pre-divided by 2*sqrt(D) so scores come out exact.  The PE accepts
    mixed-dtype matmuls (fp8e3 stationary x bf16 moving; verified on HW at
    fp32-level accuracy), so only K pays the quantization cost
    (~1.4e-2 norm rel err vs the 2e-2 gate).  V and p stay bf16.
  - Per (batch, head) unit, per 128-position block kb of the cache:
      scoresT[s,q]: lhsT = K^T block [d=128, s=128] (fp8, FWL 4x load),
                    rhs  = qt [d=128, q=16] bf16    -> psT [s=128, q=16]
      p = exp(scoresT) via ACT into p_u bf16; host-built 0/1 mask zeroes
      the <=2 tail blocks.
      out^T[dv,q] += lhsT = V block [s=128, dv=128] bf16 (natural layout),
                     rhs  = p_u block [s=128, q=16] -> accumulate in PSUM.
    Both matmuls stream only 16 columns, so the PE is weight-load bound at
    ~80 ns/block, well under the DMA rate.
  - PV runs one 16-block group behind the score stream (software
    pipelining) so the PE never head-of-line blocks on the exp.
  - Denominator: DVE strided partial sums -> ones-matmul -> [1,16]
    reciprocal -> ones[1,128]-matmul broadcast to [128,16] -> DVE mul.
    Output is written as out^T [dv=128, q=16]; the host transposes.
"""

import functools

import numpy as np
import ml_dtypes

import concourse.bacc as bacc
import concourse.mybir as mybir
import concourse.tile as tile
from concourse import bass_utils
from concourse.tile_rust import add_dep_helper


B, SQ, H, HKV, D, DV, SMAX = 4, 4, 32, 8, 128, 128, 8192
G = H // HKV  # 4 query heads per KV head
QR = SQ * G  # 16 query rows per (batch, kv-head) unit
BLK = 128  # kv positions per matmul block
GRP = 32  # blocks per PSUM score group / DMA chunk
NCORES = 8

K_DT = mybir.dt.float8e3
K_NP = np.dtype(ml_dtypes.float8_e3m4)
K_SCALE = 2.0  # K stored as e3m4(2K); Q pre-divided by 2*sqrt(D)
E3M4_MAX = 15.5
BF_DT = mybir.dt.bfloat16
BF_NP = np.dtype(ml_dtypes.bfloat16)
F32 = mybir.dt.float32


def _lean_drain_and_barrier(self, tick_clock, wait_clock):
    """Minimal TileContext exit: a single drain carrying the global-clock
    waits.  The barrier and per-semaphore clears are dropped: each kernel()
    call loads and executes the NEFF exactly once (bass2jax under axon), so
    no later execution observes the dirty semaphores."""
    from concourse.vector_clock import ScopedClock

    drain_inst = self.nc.sync.drain()
    wait_clock.add_sem_waits(
        drain_inst.ins, ScopedClock({None: tick_clock.global_clock})
    )
    popped = self.nc._tile_sem_poison_stack.pop()
    assert popped is self._sem_poison


@functools.lru_cache(maxsize=4)
def _build(nblks: tuple[int, ...]):
    """Build + compile the per-core SPMD program for given per-batch block counts."""
    nc = bacc.Bacc("TRN2", target_bir_lowering=False, debug=False)

    qt = nc.dram_tensor("qt", [D, B * QR], BF_DT, kind="ExternalInput")
    kt = [
        nc.dram_tensor(f"kt{b}", [D, n * BLK], K_DT, kind="ExternalInput")
        for b, n in enumerate(nblks)
    ]
    # V arrives host-swizzled to the SBUF image: [sl, kb*DV] with
    # v[sl, kb*DV + dv] = V[128*kb + sl, dv] — flat contiguous runs.
    v = [
        nc.dram_tensor(f"v{b}", [BLK, n * DV], BF_DT, kind="ExternalInput")
        for b, n in enumerate(nblks)
    ]
    mask = nc.dram_tensor("mask", [BLK, B * 2 * QR], BF_DT, kind="ExternalInput")
    ones = nc.dram_tensor("ones", [BLK, 1], BF_DT, kind="ExternalInput")
    ones1p = nc.dram_tensor("ones1p", [1, DV], BF_DT, kind="ExternalInput")
    out = nc.dram_tensor("out", [B, DV, QR], F32, kind="ExternalOutput")

    tile.TileContext._drain_and_barrier = _lean_drain_and_barrier
    with tile.TileContext(nc) as tc:
        with (
            tc.tile_pool(name="const", bufs=1) as cpool,
            tc.tile_pool(name="ktp", bufs=3) as ktpool,
            tc.tile_pool(name="vp", bufs=5) as vpool,
            tc.tile_pool(name="pp", bufs=2) as ppool,
            tc.tile_pool(name="small", bufs=4) as spool,
            tc.tile_pool(name="psT", bufs=3, space="PSUM") as psTpool,
            tc.tile_pool(name="psO", bufs=2, space="PSUM") as psOpool,
            tc.tile_pool(name="psDen", bufs=2, space="PSUM") as psDenpool,
            tc.tile_pool(name="psD", bufs=1, space="PSUM") as psDpool,
        ):
            # qt leads the K stream on the sync ring (8 KB, negligible);
            # mask/ones ride the scalar ring ahead of the V stream.
            qt_t = cpool.tile([D, B * QR], BF_DT, tag="qt")
            nc.scalar.dma_start(qt_t[:], qt[:])
            mask_t = cpool.tile([BLK, B * 2 * QR], BF_DT, tag="mask")
            nc.gpsimd.dma_start(mask_t[:], mask[:])
            ones_t = cpool.tile([BLK, 1], BF_DT, tag="ones")
            nc.gpsimd.dma_start(ones_t[:], ones[:])
            ones1p_t = cpool.tile([1, DV], BF_DT, tag="ones1p")
            nc.gpsimd.dma_start(ones1p_t[:], ones1p[:])

            # Per-batch group lists: (g0, glen) pairs.
            groups = []
            for b in range(B):
                gl = []
                for g0 in range(0, nblks[b], GRP):
                    gl.append((g0, min(GRP, nblks[b] - g0)))
                groups.append(gl)

            # --- DMA + compute, PV software-pipelined one group behind ---
            pend = None  # (b, g0, glen, vg)
            p_us = [None] * B
            outps = [None] * B
            denps = [None] * B

            def emit_pv(b, g0, glen, vg):
                nblk = nblks[b]
                for j in range(glen):
                    kb = g0 + j
                    nc.tensor.matmul(
                        outps[b][:],
                        lhsT=vg[:, j * DV : (j + 1) * DV],
                        rhs=p_us[b][:, kb * QR : (kb + 1) * QR],
                        start=(kb == 0),
                        stop=(kb == nblk - 1),
                    )
                # denominator: ones^T @ p accumulates [1, QR] in PSUM.
                # Trivial weight load (1 column); keeps the DVE out of the
                # batch-finish critical path entirely.
                for j in range(glen):
                    kb = g0 + j
                    nc.tensor.matmul(
                        denps[b][:],
                        lhsT=ones_t[:],
                        rhs=p_us[b][:, kb * QR : (kb + 1) * QR],
                        start=(kb == 0),
                        stop=(kb == nblk - 1),
                    )

            def emit_finish(b):
                """Reciprocal + broadcast + scale + store for a finished batch.
                The raw out^T copy runs as soon as the PV chain stops, so
                only recip -> bcast -> mul -> store trail the denominator."""
                out_raw = spool.tile([DV, QR], F32, tag="outraw")
                nc.vector.tensor_copy(out_raw[:], outps[b][:])
                recipT = spool.tile([1, QR], BF_DT, tag="recipT")
                with nc.allow_low_precision(reason="bf16 recip: 0.2% row scale"):
                    nc.vector.reciprocal(recipT[:], denps[b][:])
                recip_bc = psDpool.tile([DV, QR], F32, tag="recipbc")
                nc.tensor.matmul(
                    recip_bc[:], lhsT=ones1p_t[:], rhs=recipT[:], start=True, stop=True
                )
                out_sb = spool.tile([DV, QR], F32, tag="outsb")
                nc.vector.tensor_mul(out_sb[:], out_raw[:], recip_bc[:])
                nc.gpsimd.dma_start(out[b], out_sb[:])

            for b in range(B):
                nblk = nblks[b]
                outps[b] = psOpool.tile([DV, QR], F32, name="outp", tag="outp")  # out^T accumulator
                denps[b] = psDenpool.tile([1, QR], F32, name="denp", tag="denp")
                p_us[b] = ppool.tile([BLK, nblk * QR], BF_DT, name="p_u", tag="p_u")

                # One K DMA per batch (~1 MB fp8) on the sync HWDGE ring —
                # dma_start costs ~0.7 us of sequencer time regardless of
                # size, so fewer/bigger transfers keep the ring fed.  The
                # first batch is split so the first score matmuls can start
                # after only 4 blocks.
                ktg = ktpool.tile([D, nblk * BLK], K_DT, name="ktg", tag="ktg")
                if b == 0:
                    s0 = 0
                    for nchunk in (4, nblk - 4):
                        s1 = s0 + nchunk * BLK
                        nc.scalar.dma_start(ktg[:, s0:s1], kt[b][:, s0:s1])
                        s0 = s1
                else:
                    nc.scalar.dma_start(ktg[:], kt[b][:])

                for gi, (g0, glen) in enumerate(groups[b]):
                    # V chunk (~1 MB bf16) on the scalar HWDGE ring.  K and
                    # V queues round-robin at packet granularity, so K runs
                    # ahead of its 1:2 byte share without an explicit hold.
                    vg = vpool.tile([BLK, GRP * DV], BF_DT, tag="vg")
                    nc.sync.dma_start(
                        vg[:, : glen * DV],
                        v[b][:, g0 * DV : (g0 + glen) * DV],
                    )

                    # Scores for this group.
                    psT = psTpool.tile([BLK, GRP * QR], F32, tag="psT")  # one 2KB bank
                    for j in range(glen):
                        kb = g0 + j
                        nc.tensor.matmul(
                            psT[:, j * QR : (j + 1) * QR],
                            lhsT=ktg[:, kb * BLK : (kb + 1) * BLK],
                            rhs=qt_t[:, b * QR : (b + 1) * QR],
                            start=True,
                            stop=True,
                        )
                    nc.scalar.activation(
                        p_us[b][:, g0 * QR : (g0 + glen) * QR],
                        psT[:, : glen * QR],
                        mybir.ActivationFunctionType.Exp,
                    )
                    # zero the masked tail (lives in the last two blocks)
                    for i in range(2):
                        kb_m = nblk - 2 + i
                        if g0 <= kb_m < g0 + glen:
                            sl = slice(kb_m * QR, (kb_m + 1) * QR)
                            nc.vector.tensor_mul(
                                p_us[b][:, sl],
                                p_us[b][:, sl],
                                mask_t[:, (b * 2 + i) * QR : (b * 2 + i + 1) * QR],
                            )

                    # PV for the previous group (software pipelining).
                    if pend is not None:
                        pb, pg0, pglen, pvg = pend
                        emit_pv(pb, pg0, pglen, pvg)
                        if pb != b:
                            emit_finish(pb)
                    pend = (b, g0, glen, vg)

            # drain the pipeline
            pb, pg0, pglen, pvg = pend
            emit_pv(pb, pg0, pglen, pvg)
            emit_finish(pb)

    nc.compile()
    return nc


def _shard_inputs(Q, K, V, cache_seqlens, nblks):
    """Per-core input maps. Core c owns KV head c (query heads 4c..4c+3)."""
    qs = (np.asarray(Q, dtype=np.float32) / (K_SCALE * np.sqrt(D))).astype(BF_NP)
    K = np.asarray(K, dtype=np.float32)
    V = np.asarray(V, dtype=np.float32)
    cs = np.asarray(cache_seqlens).astype(np.int64)

    ones = np.ones((BLK, 1), np.float32).astype(BF_NP)
    ones1p = np.ones((1, DV), np.float32).astype(BF_NP)

    # 0/1 mask for the last two blocks of each batch: [128, (b, i, q)]
    mask = np.zeros((BLK, B, 2, QR), np.float32)
    sl = np.arange(BLK)
    m_of_r = np.arange(QR) // G
    for b in range(B):
        for i in range(2):
            s = (nblks[b] - 2 + i) * BLK + sl  # absolute kv position
            valid = s[:, None] <= (cs[b] - SQ + m_of_r)[None, :]
            mask[:, b, i, :] = valid.astype(np.float32)
    mask = np.ascontiguousarray(mask.reshape(BLK, B * 2 * QR)).astype(BF_NP)

    in_maps = []
    for c in range(NCORES):
        m = {
            "qt": np.ascontiguousarray(
                qs[:, :, c * G : (c + 1) * G, :].transpose(3, 0, 1, 2)
            ).reshape(D, B * QR),
            "mask": mask,
            "ones": ones,
            "ones1p": ones1p,
        }
        for b in range(B):
            nb = nblks[b]
            sb = nb * BLK
            kc = np.clip(K[b, :sb, c, :].T * K_SCALE, -E3M4_MAX, E3M4_MAX)
            m[f"kt{b}"] = np.ascontiguousarray(kc).astype(K_NP)
            # swizzle V to the SBUF block image: [sl, (kb, dv)]
            m[f"v{b}"] = np.ascontiguousarray(
                V[b, :sb, c, :].reshape(nb, BLK, DV).transpose(1, 0, 2)
            ).reshape(BLK, nb * DV).astype(BF_NP)
        in_maps.append(m)
    return in_maps


def _run(Q, K, V, cache_seqlens, trace=False, trace_cores=None):
    cs = np.asarray(cache_seqlens).astype(np.int64)
    nblks = tuple(
        int(min((int(cs[b]) + BLK - 1) // BLK, SMAX // BLK)) for b in range(B)
    )
    nc = _build(nblks)
    in_maps = _shard_inputs(Q, K, V, cache_seqlens, nblks)
    res = bass_utils.run_bass_kernel_spmd(
        nc,
        in_maps,
        core_ids=list(range(NCORES)),
        trace=trace,
        trace_cores=trace_cores,
    )
    out = np.empty((B, SQ, H, DV), np.float32)
    for c in range(NCORES):
        for b in range(B):
            # stored as out^T [dv, q]; undo on host
            out[b, :, c * G : (c + 1) * G, :] = (
                res.results[c]["out"][b].T.reshape(SQ, G, DV).astype(np.float32)
            )
    return out, res


def kernel(Q, K, V, cache_seqlens):
    out, _ = _run(Q, K, V, cache_seqlens)
    return out


# revision 19
# speedup vs baseline: 1.1540x; 1.1540x over previous
"""Trainium2 Bass kernel: GQA attention with KV cache (decode, Sq=4).

Problem shapes (hardcoded):
  Q [4, 4, 32, 128] f32, K [4, 8192, 8, 128] f32, V [4, 8192, 8, 128] f32,
  cache_seqlens [4] i32 in [4096, 8192].  Output [4, 4, 32, 128] f32.

Sharding: tensor-parallel over the 8 KV heads — core c owns KV head c and
its 4 grouped query heads, for all 4 batches.  Every core therefore does
identical work regardless of cache_seqlens skew.

v2 design (DMA-bound; ~10.96 MB/core of K+V at ~420 GB/s):
  - K is stored as fp8 e3m4 (x2 scale, clipped to +-15.5); Q is bf16 and
    pre-divided by 2*sqrt(D) so scores come out exact.  The PE accepts
    mixed-dtype matmuls (fp8e3 stationary x bf16 moving; verified on HW at
    fp32-level accuracy), so only K pays the quantization cost
    (~1.4e-2 norm rel err vs the 2e-2 gate).  V and p stay bf16.
  - Per (batch, head) unit, per 128-position block kb of the cache:
      scoresT[s,q]: lhsT = K^T block [d=128, s=128] (fp8, FWL 4x load),
                    rhs  = qt [d=128, q=16] bf16    -> psT [s=128, q=16]
      p = exp(scoresT) via ACT into p_u bf16; host-built 0/1 mask zeroes
      the <=2 tail blocks.
      out^T[dv,q] += lhsT = V block [s=128, dv=128] bf16 (natural layout),
                     rhs  = p_u block [s=128, q=16] -> accumulate in PSUM.
    Both matmuls stream only 16 columns, so the PE is weight-load bound at
    ~80 ns/block, well under the DMA rate.
  - PV runs one 16-block group behind the score stream (software
    pipelining) so the PE never head-of-line blocks on the exp.
  - Denominator: DVE strided partial sums -> ones-matmul -> [1,16]
    reciprocal -> ones[1,128]-matmul broadcast to [128,16] -> DVE mul.
    Output is written as out^T [dv=128, q=16]; the host transposes.
"""

import functools

import numpy as np
import ml_dtypes

import concourse.bacc as bacc
import concourse.mybir as mybir
import concourse.tile as tile
from concourse import bass_utils
from concourse.tile_rust import add_dep_helper


B, SQ, H, HKV, D, DV, SMAX = 4, 4, 32, 8, 128, 128, 8192
G = H // HKV  # 4 query heads per KV head
QR = SQ * G  # 16 query rows per (batch, kv-head) unit
BLK = 128  # kv positions per matmul block
GRP = 32  # blocks per PSUM score group / DMA chunk
NCORES = 8

K_DT = mybir.dt.float8e3
K_NP = np.dtype(ml_dtypes.float8_e3m4)
K_SCALE = 2.0  # K stored as e3m4(2K); Q pre-divided by 2*sqrt(D)
E3M4_MAX = 15.5
BF_DT = mybir.dt.bfloat16
BF_NP = np.dtype(ml_dtypes.bfloat16)
F32 = mybir.dt.float32


def _lean_drain_and_barrier(self, tick_clock, wait_clock):
    """Minimal TileContext exit: a single drain carrying the global-clock
    waits.  The barrier and per-semaphore clears are dropped: each kernel()
    call loads and executes the NEFF exactly once (bass2jax under axon), so
    no later execution observes the dirty semaphores."""
    from concourse.vector_clock import ScopedClock

    drain_inst = self.nc.sync.drain()
    wait_clock.add_sem_waits(
        drain_inst.ins, ScopedClock({None: tick_clock.global_clock})
    )
    popped = self.nc._tile_sem_poison_stack.pop()
    assert popped is self._sem_poison


@functools.lru_cache(maxsize=4)
def _build(nblks: tuple[int, ...]):
    """Build + compile the per-core SPMD program for given per-batch block counts."""
    nc = bacc.Bacc("TRN2", target_bir_lowering=False, debug=False)

    qt = nc.dram_tensor("qt", [D, B * QR], BF_DT, kind="ExternalInput")
    kt = [
        nc.dram_tensor(f"kt{b}", [D, n * BLK], K_DT, kind="ExternalInput")
        for b, n in enumerate(nblks)
    ]
    # V arrives host-swizzled to the SBUF image: [sl, kb*DV] with
    # v[sl, kb*DV + dv] = V[128*kb + sl, dv] — flat contiguous runs.
    v = [
        nc.dram_tensor(f"v{b}", [BLK, n * DV], BF_DT, kind="ExternalInput")
        for b, n in enumerate(nblks)
    ]
    mask = nc.dram_tensor("mask", [BLK, B * 2 * QR], BF_DT, kind="ExternalInput")
    ones = nc.dram_tensor("ones", [BLK, 1], BF_DT, kind="ExternalInput")
    ones1p = nc.dram_tensor("ones1p", [1, DV], BF_DT, kind="ExternalInput")
    out = nc.dram_tensor("out", [B, DV, QR], F32, kind="ExternalOutput")

    tile.TileContext._drain_and_barrier = _lean_drain_and_barrier
    with tile.TileContext(nc) as tc:
        with (
            tc.tile_pool(name="const", bufs=1) as cpool,
            tc.tile_pool(name="ktp", bufs=4) as ktpool,
            tc.tile_pool(name="vp", bufs=4) as vpool,
            tc.tile_pool(name="pp", bufs=4) as ppool,
            tc.tile_pool(name="small", bufs=4) as spool,
            tc.tile_pool(name="psT", bufs=3, space="PSUM") as psTpool,
            tc.tile_pool(name="psO", bufs=2, space="PSUM") as psOpool,
            tc.tile_pool(name="psDen", bufs=2, space="PSUM") as psDenpool,
            tc.tile_pool(name="psD", bufs=1, space="PSUM") as psDpool,
        ):
            # qt leads the K stream on the sync ring (8 KB, negligible);
            # mask/ones ride the scalar ring ahead of the V stream.
            qt_t = cpool.tile([D, B * QR], BF_DT, tag="qt")
            nc.sync.dma_start(qt_t[:], qt[:])
            mask_t = cpool.tile([BLK, B * 2 * QR], BF_DT, tag="mask")
            nc.gpsimd.dma_start(mask_t[:], mask[:])
            ones_t = cpool.tile([BLK, 1], BF_DT, tag="ones")
            nc.gpsimd.dma_start(ones_t[:], ones[:])
            ones1p_t = cpool.tile([1, DV], BF_DT, tag="ones1p")
            nc.gpsimd.dma_start(ones1p_t[:], ones1p[:])

            # Per-batch group lists: (g0, glen) pairs.
            groups = []
            for b in range(B):
                gl = []
                for g0 in range(0, nblks[b], GRP):
                    gl.append((g0, min(GRP, nblks[b] - g0)))
                groups.append(gl)

            # --- DMA + compute, PV software-pipelined one group behind ---
            pend = None  # (b, g0, glen, vg)
            p_us = [None] * B
            outps = [None] * B
            denps = [None] * B

            def emit_pv(b, g0, glen, vg):
                nblk = nblks[b]
                for j in range(glen):
                    kb = g0 + j
                    nc.tensor.matmul(
                        outps[b][:],
                        lhsT=vg[:, kb * DV : (kb + 1) * DV],
                        rhs=p_us[b][:, kb * QR : (kb + 1) * QR],
                        start=(kb == 0),
                        stop=(kb == nblk - 1),
                    )
                # denominator: ones^T @ p accumulates [1, QR] in PSUM.
                # Trivial weight load (1 column); keeps the DVE out of the
                # batch-finish critical path entirely.
                for j in range(glen):
                    kb = g0 + j
                    nc.tensor.matmul(
                        denps[b][:],
                        lhsT=ones_t[:],
                        rhs=p_us[b][:, kb * QR : (kb + 1) * QR],
                        start=(kb == 0),
                        stop=(kb == nblk - 1),
                    )

            def emit_finish(b):
                """Reciprocal + broadcast + scale + store for a finished batch.
                The raw out^T copy runs as soon as the PV chain stops, so
                only recip -> bcast -> mul -> store trail the denominator."""
                out_raw = spool.tile([DV, QR], F32, tag="outraw")
                nc.vector.tensor_copy(out_raw[:], outps[b][:])
                recipT = spool.tile([1, QR], BF_DT, tag="recipT")
                with nc.allow_low_precision(reason="bf16 recip: 0.2% row scale"):
                    nc.vector.reciprocal(recipT[:], denps[b][:])
                recip_bc = psDpool.tile([DV, QR], F32, tag="recipbc")
                nc.tensor.matmul(
                    recip_bc[:], lhsT=ones1p_t[:], rhs=recipT[:], start=True, stop=True
                )
                out_sb = spool.tile([DV, QR], F32, tag="outsb")
                nc.vector.tensor_mul(out_sb[:], out_raw[:], recip_bc[:])
                nc.gpsimd.dma_start(out[b], out_sb[:])

            for b in range(B):
                nblk = nblks[b]
                outps[b] = psOpool.tile([DV, QR], F32, name="outp", tag="outp")  # out^T accumulator
                denps[b] = psDenpool.tile([1, QR], F32, name="denp", tag="denp")
                p_us[b] = ppool.tile([BLK, nblk * QR], BF_DT, name="p_u", tag="p_u")

                # One K DMA per batch (~1 MB fp8) on the sync HWDGE ring —
                # dma_start costs ~0.7 us of sequencer time regardless of
                # size, so fewer/bigger transfers keep the ring fed.  The
                # first batch is split so the first score matmuls can start
                # after only 4 blocks.
                ktg = ktpool.tile([D, nblk * BLK], K_DT, name="ktg", tag="ktg")
                if b == 0:
                    s0 = 0
                    for nchunk in (4, nblk - 4):
                        s1 = s0 + nchunk * BLK
                        nc.sync.dma_start(ktg[:, s0:s1], kt[b][:, s0:s1])
                        s0 = s1
                else:
                    nc.sync.dma_start(ktg[:], kt[b][:])

                # V per batch on the scalar HWDGE ring.  The whole working
                # set (~86 KB/partition of K+V) fits in SBUF, so every tile
                # pool holds all four batches and no DMA ever waits on a
                # buffer rotation — the stream runs unthrottled at HBM rate.
                vg = vpool.tile([BLK, nblk * DV], BF_DT, name="vg", tag="vg")
                if b == 0:
                    s0 = 0
                    for nchunk in (GRP, nblk - GRP):
                        s1 = s0 + nchunk * DV
                        nc.scalar.dma_start(vg[:, s0:s1], v[b][:, s0:s1])
                        s0 = s1
                else:
                    nc.scalar.dma_start(vg[:], v[b][:])

                for gi, (g0, glen) in enumerate(groups[b]):
                    # Scores for this group.
                    psT = psTpool.tile([BLK, GRP * QR], F32, tag="psT")  # one 2KB bank
                    for j in range(glen):
                        kb = g0 + j
                        nc.tensor.matmul(
                            psT[:, j * QR : (j + 1) * QR],
                            lhsT=ktg[:, kb * BLK : (kb + 1) * BLK],
                            rhs=qt_t[:, b * QR : (b + 1) * QR],
                            start=True,
                            stop=True,
                        )
                    nc.scalar.activation(
                        p_us[b][:, g0 * QR : (g0 + glen) * QR],
                        psT[:, : glen * QR],
                        mybir.ActivationFunctionType.Exp,
                    )
                    # zero the masked tail (lives in the last two blocks)
                    for i in range(2):
                        kb_m = nblk - 2 + i
                        if g0 <= kb_m < g0 + glen:
                            sl = slice(kb_m * QR, (kb_m + 1) * QR)
                            nc.vector.tensor_mul(
                                p_us[b][:, sl],
                                p_us[b][:, sl],
                                mask_t[:, (b * 2 + i) * QR : (b * 2 + i + 1) * QR],
                            )

                    # PV for the previous group (software pipelining).
                    if pend is not None:
                        pb, pg0, pglen, pvg = pend
                        emit_pv(pb, pg0, pglen, pvg)
                        if pb != b:
                            emit_finish(pb)
                    pend = (b, g0, glen, vg)

            # drain the pipeline
            pb, pg0, pglen, pvg = pend
            emit_pv(pb, pg0, pglen, pvg)
            emit_finish(pb)

    nc.compile()
    return nc


def _shard_inputs(Q, K, V, cache_seqlens, nblks):
    """Per-core input maps. Core c owns KV head c (query heads 4c..4c+3)."""
    qs = (np.asarray(Q, dtype=np.float32) / (K_SCALE * np.sqrt(D))).astype(BF_NP)
    K = np.asarray(K, dtype=np.float32)
    V = np.asarray(V, dtype=np.float32)
    cs = np.asarray(cache_seqlens).astype(np.int64)

    ones = np.ones((BLK, 1), np.float32).astype(BF_NP)
    ones1p = np.ones((1, DV), np.float32).astype(BF_NP)

    # 0/1 mask for the last two blocks of each batch: [128, (b, i, q)]
    mask = np.zeros((BLK, B, 2, QR), np.float32)
    sl = np.arange(BLK)
    m_of_r = np.arange(QR) // G
    for b in range(B):
        for i in range(2):
            s = (nblks[b] - 2 + i) * BLK + sl  # absolute kv position
            valid = s[:, None] <= (cs[b] - SQ + m_of_r)[None, :]
            mask[:, b, i, :] = valid.astype(np.float32)
    mask = np.ascontiguousarray(mask.reshape(BLK, B * 2 * QR)).astype(BF_NP)

    in_maps = []
    for c in range(NCORES):
        m = {
            "qt": np.ascontiguousarray(
                qs[:, :, c * G : (c + 1) * G, :].transpose(3, 0, 1, 2)
            ).reshape(D, B * QR),
            "mask": mask,
            "ones": ones,
            "ones1p": ones1p,
        }
        for b in range(B):
            nb = nblks[b]
            sb = nb * BLK
            kc = np.clip(K[b, :sb, c, :].T * K_SCALE, -E3M4_MAX, E3M4_MAX)
            m[f"kt{b}"] = np.ascontiguousarray(kc).astype(K_NP)
            # swizzle V to the SBUF block image: [sl, (kb, dv)]
            m[f"v{b}"] = np.ascontiguousarray(
                V[b, :sb, c, :].reshape(nb, BLK, DV).transpose(1, 0, 2)
            ).reshape(BLK, nb * DV).astype(BF_NP)
        in_maps.append(m)
    return in_maps


def _run(Q, K, V, cache_seqlens, trace=False, trace_cores=None):
    cs = np.asarray(cache_seqlens).astype(np.int64)
    nblks = tuple(
        int(min((int(cs[b]) + BLK - 1) // BLK, SMAX // BLK)) for b in range(B)
    )
    nc = _build(nblks)
    in_maps = _shard_inputs(Q, K, V, cache_seqlens, nblks)
    res = bass_utils.run_bass_kernel_spmd(
        nc,
        in_maps,
        core_ids=list(range(NCORES)),
        trace=trace,
        trace_cores=trace_cores,
    )
    out = np.empty((B, SQ, H, DV), np.float32)
    for c in range(NCORES):
        for b in range(B):
            # stored as out^T [dv, q]; undo on host
            out[b, :, c * G : (c + 1) * G, :] = (
                res.results[c]["out"][b].T.reshape(SQ, G, DV).astype(np.float32)
            )
    return out, res


def kernel(Q, K, V, cache_seqlens):
    out, _ = _run(Q, K, V, cache_seqlens)
    return out
